# revision 24
# baseline (speedup 1.0000x reference)
"""Trainium2 Bass kernel for nn_MoEFusion (multi-modal MoE fusion MLP).

Data-parallel across 8 NeuronCores: batch dim (32768) sharded into 8
slices of 4096, all weights (<1 MB) replicated. No collectives.

v3: expert-interleaved layout (no GPSIMD broadcast, no gather DMA) +
fp8 DoubleRow W2 + paired h evictions + 1-stripe stage-2 lag.

Key layout trick (from v2): gate stationary weights are column-
replicated so partition p holds the logit of expert p%8; W1/W2 expert
weights are column/row-permuted host-side so W1 pass c computes, on
partition p, hidden unit c*16+p//8 of expert p%8. The per-expert
gating multiply is then a plain [128,512] DVE tensor_mul against the
unnormalized exp tile (2x_1P, no broadcast); the softmax denominator
is row-summed+broadcast by one ones-matmul and its reciprocal folds
into the fT eviction multiply after W2 (W2 is linear in e).

Scale bookkeeping (all cancel exactly in fT = pf * rb):
  W1' = 32 w1 (fp8), b1 rider row = 32 b1 (fp8), ACT h evict scale 8
    -> h' = 256 relu(x w1 + b1)
  e' = exp(logits + gate_b - ln 32) = e/32  (ACT bias col)
  sh' = h' * e' = 8 (h*e)  (fp8, above e4m3 denormals)
  W2' = 32 w2 (fp8 DR pairs)  -> W2 terms = 256 X1
  b2' = 8192 b2 (bf16, moving e'[0:8]) -> 256 X2
  ones = 8192 -> prs = 256 sum(e); rb = 1/(256 sum(e))
  fT = pf * rb = (sum_e e*(eo+b2))/sum(e) = fused   (exact)

DMA: features packed host-side per (modality, stripe) with contiguous
3KB per-partition rows (128 descriptors/DMA); the tiny bias tensor
loads FIRST on the scalar ring so the first x eviction is never
blocked behind the weight+feature streams.
"""

import sys

if "/opt/trn_rl_repo" not in sys.path:
    sys.path.insert(0, "/opt/trn_rl_repo")

import math
from contextlib import ExitStack

import ml_dtypes
import numpy as np

# ---- problem constants (hardcoded per contract) ----
B = 32768
NCORES = 8
BL = B // NCORES  # 4096 per core
STRIPE = 512
NSTRIPES = BL // STRIPE  # 8
NM = 3
NE = 8
D_IN = 768
KIN = D_IN // 128  # 6
D_P = 128
D_X = 384
KX = D_X // 128  # 3
NC_H = 8  # W1/W2 hidden chunks (8 chunks x 16 hidden/expert)

BF16 = ml_dtypes.bfloat16
E4M3 = ml_dtypes.float8_e4m3

WS = 32.0    # fp8 weight pre-scale (e4m3 denormal avoidance)
HS = 8.0     # extra h eviction scale (h' = 256 h)
VONES = 8192.0

# ---- fp8 packed weights (columns of [128, W8COLS]) ----
OFF_PROJ = 0                           # [p, m*768 + k*128 + o] = 32*proj_w
OFF_W1 = OFF_PROJ + NM * KIN * 128     # 2304: [p, c*512 + blk*128 + o]
#   per chunk c: [k0 | k1 | k2 | rider], rider row0 = 32*b1 permuted
OFF_GATE = OFF_W1 + NC_H * 4 * 128     # 6400: [p, kb*128+o], col o=gate_w[.,o%8]
OFF_W2F = OFF_GATE + KX * 128          # 6784: 4 DR pairs [c2*256 + blk*128 + o]
W8COLS = OFF_W2F + NC_H * 128          # 7808

# ---- bf16 packed weights (all blocks padded to full K=128/M=128) ----
OFF_PRE = 0                            # [p, 0:128] = pre_w (cols 64+ zero)
OFF_HEAD = OFF_PRE + 128               # [p, 128:130] = head_w (rows 64+ zero)
OFF_ONES = OFF_HEAD + 2                # [p, 130:258] = 8192.0 (rows 8+ zero)
OFF_B2B = OFF_ONES + 128               # [p, o] = 8192*exp_b2 (rows 8+ zero)
WBFCOLS = OFF_B2B + 128                # 386

# ---- f32 biases (columns of [128, WBCOLS]) ----
OFF_PROJB = 0
OFF_GATEB = OFF_PROJB + NM             # gate_b[p%8] - ln 32, 128 rows
OFF_PREB = OFF_GATEB + 1
OFF_HEADB = OFF_PREB + 1
WBCOLS = OFF_HEADB + 1                 # 6


def pack_weights(inp):
    p = np.arange(128)
    e_of = p % NE
    hsub = p // NE  # 0..15

    w8 = np.zeros((128, W8COLS), np.float32)
    pw = np.asarray(inp["proj_w"], np.float32) * WS
    w8[:, OFF_PROJ:OFF_W1] = (
        pw.reshape(NM, KIN, 128, 128).transpose(2, 0, 1, 3).reshape(128, -1)
    )
    # W1 permuted: pass c, stationary col o = (e=o%8, h=c*16+o//8)
    w1 = np.asarray(inp["exp_w1"], np.float32) * WS  # [NE, D_X, D_P]
    b1 = np.asarray(inp["exp_b1"], np.float32)
    blk = np.zeros((128, NC_H, 4, 128), np.float32)
    for c in range(NC_H):
        wc = w1[e_of, :, c * 16 + hsub].T  # [D_X, 128]
        blk[:, c, :KX, :] = wc.reshape(KX, 128, 128).transpose(1, 0, 2)
        blk[0, c, KX, :] = b1[e_of, c * 16 + hsub] * WS  # ones-rider row
    w8[:, OFF_W1:OFF_GATE] = blk.reshape(128, -1)
    # gate replicated: col o = gate_w[., o%8]
    gw = np.asarray(inp["gate_w"], np.float32) * WS  # [D_X, NE]
    grep = gw[:, e_of]  # [D_X, 128]
    w8[:, OFF_GATE:OFF_W2F] = grep.reshape(KX, 128, 128).transpose(1, 0, 2).reshape(
        128, -1
    )
    # W2 fp8 DR pairs: chunk pair (2j, 2j+1), row p' = (e=p'%8, h=c*16+p'//8)
    w2 = np.asarray(inp["exp_w2"], np.float32) * WS  # [NE, D_P, D_P]
    w2blk = np.zeros((128, NC_H, 128), np.float32)
    for c in range(NC_H):
        w2blk[:, c, :] = w2[e_of, c * 16 + hsub, :]
    w8[:, OFF_W2F:W8COLS] = w2blk.reshape(128, -1)
    w8 = w8.astype(E4M3)

    # pre/head padded to full partition/col counts: pre outputs M=128
    # (cols 64-127 zero), head contracts K=128 (rows 64-127 zero), and
    # the ones/b2 blocks contract K=128 (rows 8-127 zero) — full-size
    # stationary operands avoid the small-matmul PE penalty.
    wbf = np.zeros((128, WBFCOLS), np.float32)
    wbf[:, OFF_PRE:OFF_PRE + 64] = np.asarray(inp["pre_w"], np.float32)
    wbf[:64, OFF_HEAD:OFF_ONES] = np.asarray(inp["head_w"], np.float32)
    wbf[:NE, OFF_ONES:OFF_B2B] = VONES
    wbf[:NE, OFF_B2B:WBFCOLS] = np.asarray(inp["exp_b2"], np.float32) * VONES
    wbf = wbf.astype(BF16)

    wbias = np.zeros((128, WBCOLS), np.float32)
    wbias[:, OFF_PROJB:OFF_GATEB] = np.asarray(inp["proj_b"], np.float32).T
    gb = np.asarray(inp["gate_b"], np.float32)
    wbias[:, OFF_GATEB] = gb[e_of] - math.log(WS)
    wbias[:64, OFF_PREB] = np.asarray(inp["pre_b"], np.float32)
    wbias[:2, OFF_HEADB] = np.asarray(inp["head_b"], np.float32)
    return w8, wbf, wbias


def build_program(n_stripes=NSTRIPES):
    """Build the per-core Bass program (identical on all cores)."""
    import concourse.bacc as bacc
    import concourse.mybir as mybir
    import concourse.tile as tile

    f32 = mybir.dt.float32
    bf16 = mybir.dt.bfloat16
    fp8 = mybir.dt.float8e4
    AF = mybir.ActivationFunctionType
    DR = mybir.MatmulPerfMode.DoubleRow
    ALU = mybir.AluOpType
    bl = n_stripes * STRIPE

    nc = bacc.Bacc(
        "TRN2",
        target_bir_lowering=False,
        debug=False,
        enable_asserts=False,
    )

    featS = nc.dram_tensor(
        "featS", [NM, n_stripes, 128, KIN * STRIPE], fp8, kind="ExternalInput"
    ).ap()
    wmat8 = nc.dram_tensor("wmat8", [128, W8COLS], fp8, kind="ExternalInput").ap()
    wmatbf = nc.dram_tensor("wmatbf", [128, WBFCOLS], bf16, kind="ExternalInput").ap()
    wbias = nc.dram_tensor("wbias", [128, WBCOLS], f32, kind="ExternalInput").ap()
    outT = nc.dram_tensor("outT", [2, bl], f32, kind="ExternalOutput").ap()

    with tile.TileContext(nc) as tc, ExitStack() as ctx:
        wp_pool = ctx.enter_context(tc.tile_pool(name="wp", bufs=1))
        feat_pool = ctx.enter_context(tc.tile_pool(name="feat", bufs=9))
        x_pool = ctx.enter_context(tc.tile_pool(name="x", bufs=3))
        e_pool = ctx.enter_context(tc.tile_pool(name="e", bufs=3))
        r_pool = ctx.enter_context(tc.tile_pool(name="r", bufs=3))
        h_pool = ctx.enter_context(tc.tile_pool(name="h", bufs=3))
        sh_pool = ctx.enter_context(tc.tile_pool(name="sh", bufs=10))
        f_pool = ctx.enter_context(tc.tile_pool(name="f", bufs=2))
        pen_pool = ctx.enter_context(tc.tile_pool(name="pen", bufs=2))
        o_pool = ctx.enter_context(tc.tile_pool(name="o", bufs=2))

        px_pool = ctx.enter_context(tc.tile_pool(name="px", bufs=2, space="PSUM"))
        ph_pool = ctx.enter_context(tc.tile_pool(name="ph", bufs=2, space="PSUM"))
        pf_pool = ctx.enter_context(tc.tile_pool(name="pf", bufs=1, space="PSUM"))
        ps_pool = ctx.enter_context(tc.tile_pool(name="ps", bufs=1, space="PSUM"))

        # bias columns first (tiny, unblocks the first x eviction), then
        # proj weights so matmuls start early; the rest follows.
        Bz = wp_pool.tile([128, WBCOLS], f32)
        nc.scalar.dma_start(Bz[:], wbias[:])
        W8 = wp_pool.tile([128, W8COLS], fp8)
        nc.scalar.dma_start(W8[:, :OFF_W1], wmat8[:, :OFF_W1])
        Wbf = wp_pool.tile([128, WBFCOLS], bf16)

        def w8pair(off, m=128, parts=128):
            # stationary [K=128, 2, m] DoubleRow pair at col offset `off`
            return W8[:parts, off:off + 2 * m].rearrange(
                "p (two m) -> p two m", two=2
            )

        def w8s(off, n, parts=128):
            return W8[:parts, off:off + n]

        def wb(off, n, parts=128):
            return Wbf[:parts, off:off + n]

        def bslice(off, parts=128):
            return Bz[:parts, off:off + 1]

        pends = []  # (sh_pairs, eT, rb, bsl) awaiting stage-2 (lag 1)
        head_pend = None  # (pen, bsl) awaiting its head matmul

        def emit_l2_b2(eT):
            pf = pf_pool.tile([128, STRIPE], f32, tag="pf")
            nc.tensor.matmul(
                pf[:], wb(OFF_B2B, 128), eT[:],
                start=True, stop=False,
            )
            return pf

        def emit_l2_piece(pf, sht, j):
            nc.tensor.matmul(
                pf[:], w8pair(OFF_W2F + j * 256), sht[:],
                start=False, stop=(j == NC_H // 2 - 1), perf_mode=DR,
            )

        def emit_l2_fin(pf, rb):
            fT = f_pool.tile([128, STRIPE], bf16, tag="f")
            nc.vector.tensor_mul(fT[:], pf[:], rb[:])
            return fT

        def emit_l2(pend):
            sh, eT, rb, bsl = pend
            pf = emit_l2_b2(eT)
            for j in range(NC_H // 2):
                emit_l2_piece(pf, sh[j], j)
            return emit_l2_fin(pf, rb)

        def emit_pre(fT):
            pp = px_pool.tile([128, STRIPE], f32, tag="px")
            nc.tensor.matmul(pp[:], wb(OFF_PRE, 128), fT[:],
                             start=True, stop=True)
            pen = pen_pool.tile([128, STRIPE], bf16, tag="pen")
            nc.vector.tensor_scalar(
                pen[:], pp[:], bslice(OFF_PREB), 0.0,
                op0=ALU.add, op1=ALU.max,
            )
            return pen

        def emit_head2(pen, bsl):
            po = px_pool.tile([2, STRIPE], f32, tag="px")
            nc.tensor.matmul(po[:], wb(OFF_HEAD, 2), pen[:],
                             start=True, stop=True)
            ot = o_pool.tile([2, STRIPE], f32, tag="o")
            nc.scalar.activation(
                ot[:], po[:], AF.Identity, bias=bslice(OFF_HEADB, parts=2),
                scale=1.0,
            )
            nc.sync.dma_start(outT[:, bsl], ot[:])

        # ---- HAM warmup: ~40 dummy N=128 matmuls on zeros keep the PE
        # activity monitor busy during the initial DMA wait so the
        # first real matmuls run at 2.4 GHz instead of 1.2. All write
        # the same PSUM tile (same-engine WAW, no semaphore chain) and
        # the scratch is memset on GPSIMD, which is free by ~6 us. ----
        # ~70 dummies bridge PE from the barrier (~7.5 us) to the point
        # the DMA stream has delivered stripe 0 (~13 us): HAM stays hot
        # and the real matmuls never gap (a gap >3.4 us re-throttles).
        scr = wp_pool.tile([128, 2, 128], fp8)
        nc.gpsimd.memset(scr[:], 0.0)
        pd = px_pool.tile([128, 128], f32, tag="px")
        for _ in range(70):
            nc.tensor.matmul(pd[:], scr[:, 0, :], scr[:, 1, :],
                             start=True, stop=True)

        ftiles = {}

        def fetch(s, ring=None):
            if s >= n_stripes:
                return
            for m in range(NM):
                t = feat_pool.tile([128, KIN, STRIPE], fp8, tag="feat")
                eng = ring[m] if ring else nc.sync
                eng.dma_start(
                    t[:].rearrange("p k b -> p (k b)"), featS[m, s, :, :]
                )
                ftiles[(m, s)] = t

        def front(s):
            """proj -> x -> gate -> exp -> rowsum -> recip of stripe s."""
            xt = x_pool.tile([128, KX + 1, STRIPE], fp8, tag="x")
            nc.gpsimd.memset(xt[:, KX, :], 1.0)  # ones chunk (b1 rider)
            for m in range(NM):
                ft = ftiles.pop((m, s))
                px = px_pool.tile([128, STRIPE], f32, tag="px")
                for k in range(KIN // 2):
                    nc.tensor.matmul(
                        px[:],
                        w8pair(OFF_PROJ + m * KIN * 128 + k * 256),
                        ft[:, 2 * k:2 * k + 2, :],
                        start=(k == 0),
                        stop=(k == KIN // 2 - 1),
                        perf_mode=DR,
                    )
                nc.scalar.activation(
                    xt[:, m, :], px[:], AF.Identity,
                    bias=bslice(OFF_PROJB + m), scale=1.0 / WS,
                )
            # gate logits on all 128 partitions (expert p%8)
            pg = ps_pool.tile([128, STRIPE], f32, tag="ps")
            nc.tensor.matmul(
                pg[:], w8pair(OFF_GATE), xt[:, 0:2, :],
                start=True, stop=False, perf_mode=DR,
            )
            nc.tensor.matmul(
                pg[:], w8s(OFF_GATE + 256, 128), xt[:, 2, :],
                start=False, stop=True,
            )
            eT = e_pool.tile([128, STRIPE], bf16, tag="eT")
            nc.scalar.activation(
                eT[:], pg[:], AF.Exp, bias=bslice(OFF_GATEB),
                scale=1.0 / WS,
            )
            # row-sum broadcast: prs = 8192 * sum_e e'[e] = 256 sum(e)
            prs = ps_pool.tile([128, STRIPE], f32, tag="ps")
            nc.tensor.matmul(
                prs[:], wb(OFF_ONES, 128), eT[:],
                start=True, stop=True,
            )
            rb = r_pool.tile([128, STRIPE], f32, tag="rb")
            nc.vector.reciprocal_approx_fast(rb[:], prs[:])
            return xt, eT, rb

        # stripe 0: split each modality's feature DMA into k-pair chunks
        # across both rings so the first proj pass starts ~1 us earlier
        for m, eng in zip(range(NM), (nc.sync, nc.scalar, nc.sync)):
            t = feat_pool.tile([128, KIN, STRIPE], fp8, tag="feat")
            tr = t[:].rearrange("p k b -> p (k b)")
            for kp in range(KIN // 2):
                eng.dma_start(
                    tr[:, kp * 1024:(kp + 1) * 1024],
                    featS[m, 0, :, kp * 1024:(kp + 1) * 1024],
                )
            ftiles[(m, 0)] = t
        # remaining weights follow stripe 0's features in need-time
        # order at ~131-262KB granularity, so stripe-0 compute is
        # DMA-paced but continuous (no >3.4us PE gap -> HAM stays warm):
        # gate block, Wbf (prs), W1 j0j1, W1 j2j3, W2F; stripe 1 on the
        # sync ring; stripe 2 fetched inside the loop.
        nc.scalar.dma_start(W8[:, OFF_GATE:OFF_W2F], wmat8[:, OFF_GATE:OFF_W2F])
        nc.scalar.dma_start(Wbf[:], wmatbf[:])
        half_w1 = OFF_W1 + NC_H * 2 * 128
        nc.scalar.dma_start(W8[:, OFF_W1:half_w1], wmat8[:, OFF_W1:half_w1])
        nc.scalar.dma_start(W8[:, half_w1:OFF_GATE], wmat8[:, half_w1:OFF_GATE])
        nc.scalar.dma_start(W8[:, OFF_W2F:], wmat8[:, OFF_W2F:])
        fetch(1)
        cur = front(0)

        for s in range(n_stripes):
            bsl = slice(s * STRIPE, (s + 1) * STRIPE)
            xt, eT, rb = cur
            if s == 0:
                fetch(2, ring=[nc.sync, nc.scalar, nc.sync])
            fetch(s + 3)

            # head of stripe s-2, then stage-2 of stripe s-1
            if head_pend is not None:
                emit_head2(*head_pend)
                head_pend = None
            fT_prev = None
            if pends:
                p0 = pends.pop(0)
                fT_prev = emit_l2(p0)
                pend_bsl = p0[3]

            # experts: ph = W1.T x (+b1 rider); h' = 256h; sh' = 8he;
            # the next stripe's front section is pipelined into the
            # middle so its x evictions don't queue behind all 4 RELUs.
            # The last stripe also interleaves its own W2 accumulation
            # to shorten the drain chain at the end.
            last = s == n_stripes - 1
            sh = []
            pf7 = None
            for j in range(NC_H // 2):
                php = ph_pool.tile([128, 2, STRIPE], f32, tag="ph")
                for i in range(2):
                    c = 2 * j + i
                    off = OFF_W1 + c * 512
                    nc.tensor.matmul(
                        php[:, i, :], w8pair(off), xt[:, 0:2, :],
                        start=True, stop=False, perf_mode=DR,
                    )
                    nc.tensor.matmul(
                        php[:, i, :], w8pair(off + 256), xt[:, 2:4, :],
                        start=False, stop=True, perf_mode=DR,
                    )
                hp = h_pool.tile([128, 2, STRIPE], bf16, tag="h")
                nc.scalar.activation(hp[:], php[:], AF.Relu, scale=HS)
                sht = sh_pool.tile([128, 2, STRIPE], fp8, tag="sh")
                for i in range(2):
                    nc.vector.tensor_mul(sht[:, i, :], hp[:, i, :], eT[:])
                sh.append(sht)
                if j == 2 and fT_prev is not None:
                    head_pend = (emit_pre(fT_prev), pend_bsl)
                    fT_prev = None
                # stripe 0's successor front is deferred to j==3: its
                # feature tiles arrive late in the DMA-bound ramp and
                # the PE queue is FIFO — emitted earlier, front(1)'s
                # matmuls would block W1(0)'s j2/j3 groups behind them
                if j == (3 if s == 0 else 1) and not last:
                    cur = front(s + 1)
                if j == 1 and last:
                    pf7 = emit_l2_b2(eT)
                if last and j >= 2:
                    emit_l2_piece(pf7, sh[j - 2], j - 2)

            if fT_prev is not None:
                head_pend = (emit_pre(fT_prev), pend_bsl)
            if last:
                emit_l2_piece(pf7, sh[2], 2)
                emit_l2_piece(pf7, sh[3], 3)
                fT7 = emit_l2_fin(pf7, rb)
            else:
                pends.append((sh, eT, rb, bsl))

        pen7 = emit_pre(fT7)
        if head_pend is not None:
            emit_head2(*head_pend)
            head_pend = None
        emit_head2(pen7, slice((n_stripes - 1) * STRIPE, n_stripes * STRIPE))

    nc.compile()
    return nc


_PROGRAM = None


def _get_program():
    global _PROGRAM
    if _PROGRAM is None:
        _PROGRAM = build_program()
    return _PROGRAM


def make_in_maps(inputs):
    """Host-side shard + layout prep: list of 8 per-core input maps."""
    w8, wbf, wbias = pack_weights(inputs)
    feats = [
        np.asarray(inputs["feat_text"], np.float32),
        np.asarray(inputs["feat_audio"], np.float32),
        np.asarray(inputs["feat_video"], np.float32),
    ]
    in_maps = []
    for cid in range(NCORES):
        sl = slice(cid * BL, (cid + 1) * BL)
        featT = np.stack([np.ascontiguousarray(f[sl].T) for f in feats])
        # featT: [NM, 768, 4096] -> [NM, NSTRIPES(s), 128(p), KIN(k)*512(b)]
        fs = featT.reshape(NM, KIN, 128, NSTRIPES, STRIPE)
        fs = fs.transpose(0, 3, 2, 1, 4).reshape(NM, NSTRIPES, 128, KIN * STRIPE)
        in_maps.append({
            "featS": np.ascontiguousarray(fs).astype(E4M3),
            "wmat8": w8,
            "wmatbf": wbf,
            "wbias": wbias,
        })
    return in_maps


def run_on_hw(inputs, trace=False):
    from concourse.bass_utils import run_bass_kernel_spmd

    nc = _get_program()
    in_maps = make_in_maps(inputs)
    res = run_bass_kernel_spmd(
        nc, in_maps, core_ids=list(range(NCORES)), trace=trace
    )
    out = np.concatenate([r["outT"].T for r in res.results], axis=0)
    return out, res


def kernel(**inputs):
    out, _ = run_on_hw(inputs, trace=False)
    return out
